# revision 75
# baseline (speedup 1.0000x reference)
"""CARAFE upsampling kernel for 8 Trainium2 NeuronCores.

Problem (hardcoded): features (2,256,128,128) f32, masks (2,25,256,256) f32,
out (2,256,256,256) f32.  K=5, G=1, scale=2 (CARAFE content-aware upsample).

Strategy
--------
Sharding: 8 cores = batch(2) x H-half(2) x W-half(2).  Each core owns the
full C=256 and a 64x64 source patch (128x128 output patch) with a 2-pixel
feature halo (sliced with halo / zero-padded on host).

Compute: the 25-tap dynamic-filter sum becomes PSUM-accumulated TensorE
matmuls with the dy taps FOLDED INTO THE CONTRACTION DIM (the cost model
charges N columns per matmul regardless of K, so fewer/deeper matmuls cut
PE time 5/3x vs one matmul per dy).  The jj range (output column pairs,
64 per core) is split into 4 chunks of 16; each chunk contracts over
K = 40 = (2 dy-blocks) x (x-window of 20) for dy pairs {0,1} and {2,3},
plus a K=20 matmul for dy=4:

    out[c, (a jj' b)] += featT[(dyl,x'), c]^T @ band[(dyl,x'), (a jj' b)]

The feat tile stores two y-shifted copies of each x-window (dy-block 0 at
partitions 0-19 holds row y, block 1 at 20-39 holds y+1, packed host-side
at the same column), so one tile serves dy pairs {0,1} at column si,
{2,3} at si+2, and {4} at si+4 (block 0 only, K=20).  Chunks 0/1 sit at
partition base 0 and chunks 2/3 at base 64 (K=40 rounds to a 64-row PE
tile, so bases must be in {0,64}; K=20 rounds to 32, allowing the same
bases).  The 20-wide x-windows also shrink the band matrices: 100 stored
rows per output column vs 180 in a 36-wide layout.

Each (si-pair, c-half) accumulates 24 matmuls (2 si x 4 chunks x 3
dy-groups, N=64 each) into one full PSUM bank [128,512].  The matmuls
write a strided (a,32) PSUM view so the bank is already in output
(s,a,j) order and the drain is a plain 1-D copy (engines only support
3 free dims).  Copies alternate DVE/Act (GPSIMD cannot read PSUM).
Output leaves as fp16 (rel tol is 2e-2; fp16 adds ~5e-4) and the host
upcasts, halving the dominant store traffic.

DMA queue assignment (measured on the TimelineSim contention model):
band-pair + feat loads on SP (sync), dy=4 band loads on the GPSIMD SWDGE
queue (skips the serial HWDGE), stores on GPSIMD.  The last group's
stores are split in half on the idle SP queue: the scalar queue would
park them behind the final Act copy (in-order SEQ), and gpsimd would
serialize desc-gen into the drain.  Group 0's first loads are split
small and interleaved in PE-consumption order (feat first - its DGE
latency hides the band issue) so matmuls start ~3.6 us in.

TimelineSim: 52432 ns/core (baseline 82276).  DMA engines busy 47.9 us
(out 23.3 + feat 15.5 + bands 9.1) with ~3 us idle; PE busy ~43 us
(start 3.6 + busy + 4.4 drain); both paths converge on the wall, so
further gains need simultaneous feat-byte AND PE-column cuts - folding
deeper trades 7.7 us DMA per 13.7 us PE and never pays at this balance.
"""

import numpy as np

import concourse.bacc as bacc
import concourse.bass as bass
import concourse.mybir as mybir
import concourse.tile as tile
from concourse.bass_utils import run_bass_kernel_spmd

FP16 = mybir.dt.float16
F32 = mybir.dt.float32

N_CORES = 8
C = 256
SI = 64          # source row indices per core
XW = 20          # x-window per jj-chunk (16 + 4 halo)
KP = 40          # contraction partitions per dy-pair matmul (2 x XW)
YR = 68          # y rows incl. halo
NG = 8           # si groups
GROUP = 8        # si per group

_CACHED_NC = None
TRACE = False
_LAST_RESULTS = None

NSB = NG // 2    # band superblocks (2 si-groups each)
GR2 = 2 * GROUP  # si per superblock


def _build_nc():
    nc = bacc.Bacc(None, target_bir_lowering=False, debug=False)

    # [slice(p-base 0/64), p=(dyl,x'), y, tidx, c]; chunk = 2*slice + tidx
    featT_d = nc.dram_tensor("featT", [2, KP, YR, 2, C], FP16, kind="ExternalInput")
    # dy pairs {0,1},{2,3}: [g, slice, p=(dyl,x'), sl, (tidx grp n)]
    bands_d = nc.dram_tensor("bands", [NG, 2, KP, GROUP, 2 * 2 * 64], FP16,
                             kind="ExternalInput")
    # dy=4: [g, slice, p=x', sl, (tidx n)]
    bands2_d = nc.dram_tensor("bands2", [NG, 2, XW, GROUP, 2 * 64], FP16,
                              kind="ExternalInput")
    out_d = nc.dram_tensor("out", [C, 2 * SI, 2 * SI], FP16, kind="ExternalOutput")

    with tile.TileContext(nc) as tc:
        with (
            tc.tile_pool(name="feat", bufs=1) as fpool,
            tc.tile_pool(name="bands", bufs=3) as bpool,
            tc.tile_pool(name="psum", bufs=8, space=bass.MemorySpace.PSUM) as ppool,
            tc.tile_pool(name="stage", bufs=4) as spool,
        ):
            # col = y*512 + tidx*256 + c
            ftile = fpool.tile([104, YR * 2 * C], FP16, tag="ft", name="ft")

            # GPSIMD cannot read PSUM, so copies go to DVE + Act only
            copy_engines = [nc.vector.tensor_copy, nc.scalar.copy]
            ncopy = 1

            ydone = 0

            def load_feat_rows(upto):
                nonlocal ydone
                upto = min(upto, YR)
                if upto <= ydone:
                    return
                for s in range(2):
                    nc.sync.dma_start(
                        ftile[64 * s : 64 * s + KP, ydone * 512 : upto * 512],
                        featT_d[s, :, ydone:upto].rearrange("p y t c -> p (y t c)"),
                    )
                ydone = upto

            btiles = {}

            def load_band_group(g):
                if g >= NG or g in btiles:
                    return
                # band cols per si: tidx*128 + grp*64 + n  (dy pairs)
                bt = bpool.tile([104, GROUP * 256], FP16, tag="bt", name="bt")
                # dy=4 cols per si: tidx*64 + n
                b2t = bpool.tile([104, GROUP * 128], FP16, tag="b2", name="b2")
                btiles[g] = (bt, b2t)
                for s in range(2):
                    nc.sync.dma_start(
                        bt[64 * s : 64 * s + KP, :],
                        bands_d[g, s].rearrange("p sl x -> p (sl x)"),
                    )
                    nc.gpsimd.dma_start(
                        b2t[64 * s : 64 * s + XW, :],
                        bands2_d[g, s].rearrange("p sl x -> p (sl x)"),
                    )

            for g in range(NG):
                if g == 0:
                    btile = bpool.tile([104, GROUP * 256], FP16, tag="bt", name="bt")
                    btile2 = bpool.tile([104, GROUP * 128], FP16, tag="b2", name="b2")
                    btiles[0] = (btile, btile2)
                else:
                    load_band_group(g)  # no-op when prefetched by g-1
                    btile, btile2 = btiles[g]
                if g == 0:
                    # split first loads so matmuls can start early; the tiny
                    # b2 slices go right after each band slice so chain k0
                    # never waits on them
                    for s in range(2):
                        nc.sync.dma_start(
                            ftile[64 * s : 64 * s + KP, : 6 * 512],
                            featT_d[s, :, :6].rearrange("p y t c -> p (y t c)"),
                        )
                        nc.sync.dma_start(
                            btile[64 * s : 64 * s + KP, : 2 * 256],
                            bands_d[0, s, :, :2].rearrange("p sl x -> p (sl x)"),
                        )
                        nc.gpsimd.dma_start(
                            btile2[64 * s : 64 * s + XW, : 2 * 128],
                            bands2_d[0, s, :, :2].rearrange("p sl x -> p (sl x)"),
                        )
                    ydone = 6
                    for s in range(2):
                        nc.sync.dma_start(
                            ftile[64 * s : 64 * s + KP, 6 * 512 : 13 * 512],
                            featT_d[s, :, 6:13].rearrange("p y t c -> p (y t c)"),
                        )
                        nc.sync.dma_start(
                            btile[64 * s : 64 * s + KP, 2 * 256 :],
                            bands_d[0, s, :, 2:].rearrange("p sl x -> p (sl x)"),
                        )
                        nc.gpsimd.dma_start(
                            btile2[64 * s : 64 * s + XW, 2 * 128 :],
                            bands2_d[0, s, :, 2:].rearrange("p sl x -> p (sl x)"),
                        )
                    ydone = 13
                else:
                    load_feat_rows(8 * g + 12)
                    load_band_group(g + 1)
                    load_feat_rows(8 * g + 20)
                sloff = 0
                for ch in range(2):
                    stg = spool.tile([128, GROUP * 256], FP16)
                    for k in range(4):  # si-pairs in group
                        ps = ppool.tile([128, 512], F32)
                        # psum col = (2*sl2 + a)*128 + t4*32 + jj'*2 + b: the
                        # matmul writes a strided (a, 32) view so PSUM is
                        # already in output (s, a, j) order and the drain
                        # copy below is a plain 1-D copy
                        pview = ps[:].rearrange("p (u t j) -> p u t j",
                                                u=4, t=4, j=32)
                        for sl2 in range(2):
                            sl = 2 * k + sl2
                            si = GROUP * g + sl
                            for t4 in range(4):  # jj chunk
                                s, t = t4 // 2, t4 % 2
                                pb = 64 * s
                                pdst = pview[:, 2 * sl2 : 2 * sl2 + 2, t4, :]
                                for grp in range(2):
                                    lcol = (si + 2 * grp) * 512 + t * 256 + ch * 128
                                    bcol = (sloff + sl) * 256 + t * 128 + grp * 64
                                    nc.tensor.matmul(
                                        pdst,
                                        ftile[pb : pb + KP, lcol : lcol + 128],
                                        btile[pb : pb + KP, bcol : bcol + 64],
                                        start=(sl2 == 0 and t4 == 0 and grp == 0),
                                        stop=False,
                                        skip_group_check=True,
                                    )
                                lcol = (si + 4) * 512 + t * 256 + ch * 128
                                b2col = (sloff + sl) * 128 + t * 64
                                nc.tensor.matmul(
                                    pdst,
                                    ftile[pb : pb + XW, lcol : lcol + 128],
                                    btile2[pb : pb + XW, b2col : b2col + 64],
                                    start=False,
                                    stop=(sl2 == 1 and t4 == 3),
                                    skip_group_check=True,
                                )
                        copy_engines[ncopy % 2](
                            stg[:, k * 512 : (k + 1) * 512], ps[:])
                        ncopy += 1
                    odst = out_d[ch * 128 : (ch + 1) * 128,
                                 g * 2 * GROUP : (g + 1) * 2 * GROUP, :]
                    if g == NG - 1:
                        # split the drain-tail stores on the idle SP queue:
                        # the scalar queue would park them behind the final
                        # Act copy (in-order SEQ)
                        nc.sync.dma_start(
                            odst[:, : GROUP].rearrange("c i j -> c (i j)"),
                            stg[:, : GROUP * 128],
                        )
                        nc.sync.dma_start(
                            odst[:, GROUP :].rearrange("c i j -> c (i j)"),
                            stg[:, GROUP * 128 :],
                        )
                    else:
                        nc.gpsimd.dma_start(
                            odst.rearrange("c i j -> c (i j)"), stg[:]
                        )

    nc.compile()
    return nc


def _get_nc():
    global _CACHED_NC
    if _CACHED_NC is None:
        _CACHED_NC = _build_nc()
    return _CACHED_NC


def _prep_core_inputs(features: np.ndarray, masks: np.ndarray):
    fp = np.pad(features, ((0, 0), (0, 0), (2, 2), (2, 2)))

    jl = np.arange(16)
    in_maps = []
    for core in range(N_CORES):
        n, hb, wb = core // 4, (core // 2) % 2, core % 2

        fsl = fp[n, :, hb * SI : hb * SI + YR, wb * SI : wb * SI + YR]
        fT = np.ascontiguousarray(fsl.transpose(2, 1, 0))  # (x, y, c)
        fTp = np.pad(fT, ((0, 0), (0, 1), (0, 0)))  # y to 69 for dy-block 1
        featT = np.empty((2, KP, YR, 2, C), dtype=np.float16)
        for s in range(2):
            for t in range(2):
                x0 = 16 * (2 * s + t)
                for dyl in range(2):
                    featT[s, 20 * dyl : 20 * dyl + 20, :, t, :] = \
                        fTp[x0 : x0 + 20, dyl : dyl + YR, :]

        msl = masks[n, :, hb * 2 * SI : (hb + 1) * 2 * SI,
                    wb * 2 * SI : (wb + 1) * 2 * SI]
        m6 = msl.reshape(5, 5, SI, 2, SI, 2)  # dy dx si a jj b
        bh = np.zeros((NG, 2, KP, GROUP, 2, 2, 2, 16, 2), dtype=np.float32)
        bh2 = np.zeros((NG, 2, XW, GROUP, 2, 2, 16, 2), dtype=np.float32)
        for s in range(2):
            for t in range(2):
                chunk = 2 * s + t
                for dx in range(5):
                    for grp in range(2):
                        for dyl in range(2):
                            dy = 2 * grp + dyl
                            # m: (g, sl, a, jl, b) -> rows jl+dx
                            m = m6[dy, dx].reshape(NG, GROUP, 2, 4, 16, 2)[
                                :, :, :, chunk, :, :]
                            bh[:, s, 20 * dyl + jl + dx, :, t, grp, :, jl, :] = \
                                m.transpose(3, 0, 1, 2, 4)
                    m = m6[4, dx].reshape(NG, GROUP, 2, 4, 16, 2)[
                        :, :, :, chunk, :, :]
                    bh2[:, s, jl + dx, :, t, :, jl, :] = m.transpose(3, 0, 1, 2, 4)
        bands = bh.astype(np.float16).reshape(NG, 2, KP, GROUP, 256)
        bands2 = bh2.astype(np.float16).reshape(NG, 2, XW, GROUP, 128)

        in_maps.append({"featT": featT, "bands": bands, "bands2": bands2})
    return in_maps


def kernel(features: np.ndarray, masks: np.ndarray) -> np.ndarray:
    global _LAST_RESULTS
    features = np.asarray(features, dtype=np.float32)
    masks = np.asarray(masks, dtype=np.float32)

    nc = _get_nc()
    in_maps = _prep_core_inputs(features, masks)
    res = run_bass_kernel_spmd(nc, in_maps, list(range(N_CORES)), trace=TRACE)
    _LAST_RESULTS = res

    out = np.empty((2, C, 256, 256), dtype=np.float32)
    for core in range(N_CORES):
        n, hb, wb = core // 4, (core // 2) % 2, core % 2
        out[n, :, hb * 128 : (hb + 1) * 128, wb * 128 : (wb + 1) * 128] = \
            res.results[core]["out"].astype(np.float32)
    return out
